# revision 1
# baseline (speedup 1.0000x reference)
import numpy as np
import jax
import jax.numpy as jnp
from functools import partial

# nn_GaussianAttention: B=64, T=512, H=1024, K=10, U=128, C=128, D=3
# Sharding: data-parallel over batch across 8 cores; per-batch params
# (init_kappa, char_seq) shard on batch too; window_w/b replicated.
# The cumsum over time stays local per device.

N_CORES = 8


def _gaussian_attention(input0, original, init_kappa, char_seq, window_w, window_b):
    B, T, H = input0.shape
    K = init_kappa.shape[1]
    U = char_seq.shape[1]
    abk = jnp.exp(input0 @ window_w + window_b).reshape(B, T, 3, K)
    alpha = abk[:, :, 0, :]
    beta = abk[:, :, 1, :]
    kappa_inc = abk[:, :, 2, :]
    kappa = init_kappa[:, None, :, 0] + jnp.cumsum(kappa_inc, axis=1)  # [B,T,K]
    u = jnp.arange(U, dtype=input0.dtype)
    diff2 = (kappa[..., None] - u) ** 2                                # [B,T,K,U]
    phi = jnp.sum(alpha[..., None] * jnp.exp(-beta[..., None] * diff2), axis=2)
    window = jnp.einsum('btu,buc->btc', phi, char_seq)
    return jnp.concatenate([input0, window, original], axis=-1)


def kernel(input0, original, init_kappa, char_seq, window_w, window_b):
    input0 = np.asarray(input0, dtype=np.float32)
    original = np.asarray(original, dtype=np.float32)
    init_kappa = np.asarray(init_kappa, dtype=np.float32)
    char_seq = np.asarray(char_seq, dtype=np.float32)
    window_w = np.asarray(window_w, dtype=np.float32)
    window_b = np.asarray(window_b, dtype=np.float32)

    B = input0.shape[0]
    devs = jax.devices()
    n = N_CORES if len(devs) >= N_CORES and B % N_CORES == 0 else 1

    if n > 1:
        bs = B // n

        def shard(x):
            return x.reshape((n, bs) + x.shape[1:])

        f = jax.pmap(
            lambda i0, orig, ik, cs, ww, wb: _gaussian_attention(i0, orig, ik, cs, ww, wb),
            axis_name='x', devices=devs[:n],
        )
        out = f(
            shard(input0), shard(original), shard(init_kappa), shard(char_seq),
            np.broadcast_to(window_w, (n,) + window_w.shape),
            np.broadcast_to(window_b, (n,) + window_b.shape),
        )
        out = np.asarray(out).reshape((B,) + out.shape[2:])
    else:
        out = np.asarray(jax.jit(_gaussian_attention)(
            input0, original, init_kappa, char_seq, window_w, window_b))
    return out.astype(np.float32)

